# revision 27
# baseline (speedup 1.0000x reference)
"""Block-DCT quantizer (8x8 DCT -> quant/dequant -> IDCT) on 8 Trainium2 cores.

Sharding: pure data parallel over batch (core b processes x[b] = [3,1024,1024])
with a host-chosen vec-block layout: each 8x8 block is one 64-vector along the
SBUF partition dim (two blocks stacked -> 128 partitions), columns enumerate
blocks. In this layout the full 2D DCT is ONE 128x128 block-diagonal matmul
with kron(D,D) per 64-group -- no on-device transposes at all (the DVE
32x32-block transposes that dominated the previous version are gone):

    S_F  ps1 = (A^T/qstep stacked)^T @ x        (PE, bf16 in / fp32 PSUM)
    R    q   = bf16(ps1 + 192)                  (bf16 ulp=1 in [128,256), so
                                                 this rounds c/qstep to the
                                                 nearest integer, offset by
                                                 192, exactly for |q| < 64)
    S_I  ps2 = (qstep*A stacked)^T @ q          (PE)
    E4   o   = ps2 + corr                       (corr = per-partition [128,1])
    out  DMA o (bf16) -> host gathers + casts fp32

corr is a device-computed probe MM(lhsT_I, const(-192)), bitwise equal to the
magic offset's contribution in ps2, so for q==0 blocks the output is exactly
0.0, matching the fp32 reference exactly in the qp=32 regime.

R and E4 are each one PSUM->SBUF instruction per [128,1024] chunk (PSUM tiles
span 2 banks) and alternate between the Scalar and Vector engines by chunk
parity so both engines carry half the elementwise load. I/O is bf16 both ways
(the device math was bf16 already; the cast is part of host-side shard
marshaling), so per-core HBM traffic is 2 x 6.29 MB. DMA moves [128, 4096]
tiles (8 KB contiguous per partition line); input transfers trigger from the
Sync queue and output transfers from the GpSimd queue so neither stream's
semaphore waits block the other. The chunk pipeline is emitted
software-pipelined, deepest stage first, so each engine's in-order queue
interleaves chunks.
"""
import math
import sys

sys.path.insert(0, "/opt/trn_rl_repo")

import ml_dtypes
import numpy as np

import concourse.bass as bass  # noqa: F401
import concourse.mybir as mybir
import concourse.tile as tile
from concourse import bacc, bass_utils

P = 128
MW = 512          # matmul free dim = one PSUM bank of fp32
CW = 1024         # compute chunk = 2 PSUM banks; R/E4 run once per chunk
DW = 4096         # DMA tile width (4 chunks per transfer, 8KB/partition lines)
N_CORES = 8
MAGIC = 192.0     # bf16 round-to-int bias: ulp(bf16)=1 on [128, 256)

_BUILD_CACHE = {}


def _dct_matrix(n: int) -> np.ndarray:
    k = np.arange(n, dtype=np.float64)[:, None]
    j = np.arange(n, dtype=np.float64)[None, :]
    d = np.cos(math.pi / n * (j + 0.5) * k)
    scale = np.full((n, 1), math.sqrt(2.0 / n))
    scale[0, 0] = math.sqrt(1.0 / n)
    return d * scale


def _build(ftot: int):
    key = (ftot,)
    if key in _BUILD_CACHE:
        return _BUILD_CACHE[key]

    assert ftot % DW == 0
    n_chunks = ftot // CW
    grp = DW // CW
    f32 = mybir.dt.float32
    bf16 = mybir.dt.bfloat16

    nc = bacc.Bacc("TRN2", target_bir_lowering=False, debug=False,
                   num_devices=N_CORES)
    x = nc.dram_tensor("x", [P, ftot], bf16, kind="ExternalInput").ap()
    wts = nc.dram_tensor("wts", [P, 2 * P + 1], bf16,
                         kind="ExternalInput").ap()
    y = nc.dram_tensor("y", [P, ftot], bf16, kind="ExternalOutput").ap()
    HB = P // 2   # 64: contraction depth of one block-diagonal group

    with tile.TileContext(nc) as tc:
        with tc.tile_pool(name="consts", bufs=1) as cpool, \
             tc.tile_pool(name="io", bufs=3) as iopool, \
             tc.tile_pool(name="mid", bufs=4) as midpool, \
             tc.tile_pool(name="psum", bufs=4, space="PSUM") as psum:
            wtile = cpool.tile([P, 2 * P + 1], bf16, tag="wts",
                               name="wtile")
            nc.sync.dma_start(out=wtile, in_=wts)
            w_neg = wtile[:, 2 * P:2 * P + 1]   # constant -MAGIC column

            def mm_pair(out, w0, rhs, noload=False):
                mi = nc.tensor.matmul(out, lhsT=wtile[:, w0:w0 + P], rhs=rhs,
                                      start=True, stop=True)
                if noload:
                    # stationary already resident from the previous matmul
                    mi.ins.ldweights = False
                return mi

            # Device-side probe: corr = (qstep*A)^T @ (-MAGIC), bitwise equal
            # to the magic offset's contribution in ps2, so E4 cancels it
            # exactly (all-zero blocks come out as exact 0.0).
            psc = psum.tile([P, 1], f32, tag="ps", name="psc")
            mm_pair(psc, P, w_neg)
            corr = cpool.tile([P, 1], f32, tag="corr", name="corr")
            nc.vector.tensor_copy(out=corr, in_=psc)

            st = [dict() for _ in range(n_chunks)]
            ident = mybir.ActivationFunctionType.Identity

            # input groups: two half-size leading transfers so the first
            # matmul starts ~1.4us earlier; output groups: two half-size
            # trailing transfers so the final flush is ~1.4us shorter
            def spans(sizes):
                table, c = {}, 0
                for s in sizes:
                    for j in range(s):
                        table[c + j] = (c, s)
                    c += s
                assert c == n_chunks
                return table

            in_start = spans([grp // 2, grp // 2]
                             + [grp] * (n_chunks // grp - 1))
            out_start = spans([grp] * (n_chunks // grp - 1)
                              + [grp // 2, grp // 2])

            def stage(k, i):
                v = st[i]
                g, sl = i // grp, i % grp
                c0i, siz = in_start[i]
                if k == 0:
                    if i == c0i:
                        v["xt"] = iopool.tile([P, siz * CW], bf16, tag="xt",
                                              bufs=4, name="xt")
                        nc.sync.dma_start(
                            out=v["xt"],
                            in_=x[:, c0i * CW:(c0i + siz) * CW])
                elif k == 2:
                    xt = st[c0i]["xt"]
                    v["ps1"] = psum.tile([P, CW], f32, tag="ps", name="ps")
                    for m in range(CW // MW):
                        c0 = (i - c0i) * CW + m * MW
                        mm_pair(v["ps1"][:, m * MW:(m + 1) * MW], 0,
                                xt[:, c0:c0 + MW], noload=m > 0)
                    if i == c0i + siz - 1:
                        st[c0i].pop("xt")
                elif k == 3:
                    v["q"] = midpool.tile([P, CW], bf16, tag="q", name="q")
                    # round: bf16(v + 192) == round(v) + 192 for |v| < 64
                    if i % 2 == 0:
                        nc.scalar.activation(v["q"], v.pop("ps1"),
                                             mybir.ActivationFunctionType.Copy,
                                             bias=MAGIC)
                    else:
                        nc.vector.tensor_scalar_add(v["q"], v.pop("ps1"),
                                                    MAGIC)
                elif k == 4:
                    v["ps2"] = psum.tile([P, CW], f32, tag="ps", name="ps")
                    for m in range(CW // MW):
                        mm_pair(v["ps2"][:, m * MW:(m + 1) * MW], P,
                                v["q"][:, m * MW:(m + 1) * MW], noload=m > 0)
                    v.pop("q")
                elif k == 5:
                    c0o, so = out_start[i]
                    if i == c0o:
                        v["ot"] = iopool.tile([P, so * CW], bf16, tag="ot",
                                              name="ot")
                    ot = st[c0o]["ot"]
                    dst = ot[:, (i - c0o) * CW:(i - c0o + 1) * CW]
                    if i % 2 == 0:
                        nc.vector.tensor_scalar_add(dst, v.pop("ps2"), corr)
                    else:
                        nc.scalar.activation(dst, v.pop("ps2"), ident,
                                             bias=corr)
                elif k == 6:
                    c0o, so = out_start[i]
                    if i == c0o + so - 1:
                        nc.gpsimd.dma_start(
                            out=y[:, c0o * CW:(c0o + so) * CW],
                            in_=st[c0o].pop("ot"))

            n_stages = 7
            for t in range(n_chunks + n_stages - 1):
                for k in range(n_stages - 1, -1, -1):  # deepest stage first
                    i = t - k
                    if 0 <= i < n_chunks:
                        stage(k, i)

    nc.compile()
    _BUILD_CACHE[key] = nc
    return nc


def kernel(x: np.ndarray, block_size, qp, _trace: bool = False,
           _results_out: list | None = None) -> np.ndarray:
    n = int(block_size)
    qp = int(qp)
    b, ch, h, w = x.shape
    assert b == N_CORES, f"expected batch {N_CORES}, got {b}"
    assert h % n == 0 and w % n == 0, "padding path not implemented"
    nn = n * n
    assert P % nn == 0, f"block size {n}: {nn} must divide {P}"
    G = P // nn                      # blocks stacked per partition column
    nb = ch * (h // n) * (w // n)    # blocks per core
    assert nb % G == 0
    ftot = nb // G
    assert ftot % DW == 0

    qstep = float(np.float32(2.0 ** ((qp - 4.0) / 6.0)))
    d = _dct_matrix(n)
    a = np.kron(d, d)                # [nn, nn]: coeff = a @ vec(block)
    eye_g = np.eye(G)
    wts_np = np.zeros((P, 2 * P + 1), dtype=np.float64)
    wts_np[:, 0:P] = np.kron(eye_g, a.T / qstep)   # forward-DCT stationary
    wts_np[:, P:2 * P] = np.kron(eye_g, qstep * a)  # inverse-DCT stationary
    wts_np[:, 2 * P] = -MAGIC
    consts = {"wts": np.ascontiguousarray(
        wts_np.astype(ml_dtypes.bfloat16))}

    nc = _build(ftot)

    x_np = np.asarray(x, dtype=np.float32)
    in_maps = []
    for i in range(N_CORES):
        arr = x_np[i].reshape(ch, h // n, n, w // n, n)
        arr = arr.transpose(2, 4, 0, 1, 3).reshape(nn, nb)
        arr = arr.reshape(nn, G, ftot).transpose(1, 0, 2).reshape(P, ftot)
        in_maps.append(
            {"x": np.ascontiguousarray(arr.astype(ml_dtypes.bfloat16)),
             **consts})

    res = bass_utils.run_bass_kernel_spmd(
        nc, in_maps, core_ids=list(range(N_CORES)), trace=_trace)
    if _results_out is not None:
        _results_out.append(res)

    out = np.empty((b, ch, h, w), dtype=np.float32)
    for i in range(N_CORES):
        yd = res.results[i]["y"].astype(np.float32)
        arr = yd.reshape(G, nn, ftot).transpose(1, 0, 2).reshape(nn, nb)
        arr = arr.reshape(n, n, ch, h // n, w // n)
        out[i] = arr.transpose(2, 3, 0, 4, 1).reshape(ch, h, w)
    return out


# revision 28
# speedup vs baseline: 1.0750x; 1.0750x over previous
"""Block-DCT quantizer (8x8 DCT -> quant/dequant -> IDCT) on 8 Trainium2 cores.

Sharding: pure data parallel over batch (core b processes x[b] = [3,1024,1024])
with a host-chosen vec-block layout: each 8x8 block is one 64-vector along the
SBUF partition dim (two blocks stacked -> 128 partitions), columns enumerate
blocks. In this layout the full 2D DCT is ONE 128x128 block-diagonal matmul
with kron(D,D) per 64-group -- no on-device transposes at all (the DVE
32x32-block transposes that dominated the previous version are gone):

    S_F  ps1 = (A^T/qstep stacked)^T @ x        (PE, bf16 in / fp32 PSUM)
    R    q   = bf16(ps1 + 192)                  (bf16 ulp=1 in [128,256), so
                                                 this rounds c/qstep to the
                                                 nearest integer, offset by
                                                 192, exactly for |q| < 64)
    S_I  ps2 = (qstep*A stacked)^T @ q          (PE)
    E4   o   = ps2 + corr                       (corr = per-partition [128,1])
    out  DMA o (bf16) -> host gathers + casts fp32

corr is a device-computed probe MM(lhsT_I, const(-192)), bitwise equal to the
magic offset's contribution in ps2, so for q==0 blocks the output is exactly
0.0, matching the fp32 reference exactly in the qp=32 regime.

R and E4 are each one PSUM->SBUF instruction per [128,1024] chunk (PSUM tiles
span 2 banks) and alternate between the Scalar and Vector engines by chunk
parity so both engines carry half the elementwise load. I/O is bf16 both ways
(the device math was bf16 already; the cast is part of host-side shard
marshaling), so per-core HBM traffic is 2 x 6.29 MB. DMA moves [128, 4096]
tiles (8 KB contiguous per partition line); input transfers trigger from the
Sync queue and output transfers from the GpSimd queue so neither stream's
semaphore waits block the other. The chunk pipeline is emitted
software-pipelined, deepest stage first, so each engine's in-order queue
interleaves chunks.
"""
import math
import sys

sys.path.insert(0, "/opt/trn_rl_repo")

import ml_dtypes
import numpy as np

import concourse.bass as bass  # noqa: F401
import concourse.mybir as mybir
import concourse.tile as tile
from concourse import bacc, bass_utils

P = 128
MW = 512          # matmul free dim = one PSUM bank of fp32
CW = 1024         # compute chunk = 2 PSUM banks; R/E4 run once per chunk
DW = 4096         # DMA tile width (4 chunks per transfer, 8KB/partition lines)
N_CORES = 8
MAGIC = 192.0     # bf16 round-to-int bias: ulp(bf16)=1 on [128, 256)

_BUILD_CACHE = {}


def _dct_matrix(n: int) -> np.ndarray:
    k = np.arange(n, dtype=np.float64)[:, None]
    j = np.arange(n, dtype=np.float64)[None, :]
    d = np.cos(math.pi / n * (j + 0.5) * k)
    scale = np.full((n, 1), math.sqrt(2.0 / n))
    scale[0, 0] = math.sqrt(1.0 / n)
    return d * scale


def _build(ftot: int):
    key = (ftot,)
    if key in _BUILD_CACHE:
        return _BUILD_CACHE[key]

    assert ftot % DW == 0
    n_chunks = ftot // CW
    grp = DW // CW
    f32 = mybir.dt.float32
    bf16 = mybir.dt.bfloat16

    nc = bacc.Bacc("TRN2", target_bir_lowering=False, debug=False,
                   num_devices=N_CORES)
    x = nc.dram_tensor("x", [P, ftot], bf16, kind="ExternalInput").ap()
    wts = nc.dram_tensor("wts", [P, 2 * P + 1], bf16,
                         kind="ExternalInput").ap()
    y = nc.dram_tensor("y", [P, ftot], bf16, kind="ExternalOutput").ap()
    HB = P // 2   # 64: contraction depth of one block-diagonal group

    with tile.TileContext(nc) as tc:
        with tc.tile_pool(name="consts", bufs=1) as cpool, \
             tc.tile_pool(name="io", bufs=3) as iopool, \
             tc.tile_pool(name="mid", bufs=4) as midpool, \
             tc.tile_pool(name="psum", bufs=4, space="PSUM") as psum:
            wtile = cpool.tile([P, 2 * P + 1], bf16, tag="wts",
                               name="wtile")
            nc.sync.dma_start(out=wtile, in_=wts)
            w_neg = wtile[:, 2 * P:2 * P + 1]   # constant -MAGIC column

            def mm_pair(out, w0, rhs, noload=False):
                mi = nc.tensor.matmul(out, lhsT=wtile[:, w0:w0 + P], rhs=rhs,
                                      start=True, stop=True)
                if noload:
                    # stationary already resident from the previous matmul
                    mi.ins.ldweights = False
                return mi

            # Device-side probe: corr = (qstep*A)^T @ (-MAGIC), bitwise equal
            # to the magic offset's contribution in ps2, so E4 cancels it
            # exactly (all-zero blocks come out as exact 0.0).
            psc = psum.tile([P, 1], f32, tag="ps", name="psc")
            mm_pair(psc, P, w_neg)
            corr = cpool.tile([P, 1], f32, tag="corr", name="corr")
            nc.vector.tensor_copy(out=corr, in_=psc)

            st = [dict() for _ in range(n_chunks)]
            ident = mybir.ActivationFunctionType.Identity

            # input groups: two half-size leading transfers so the first
            # matmul starts ~1.4us earlier; output groups: two half-size
            # trailing transfers so the final flush is ~1.4us shorter
            def spans(sizes):
                table, c = {}, 0
                for s in sizes:
                    for j in range(s):
                        table[c + j] = (c, s)
                    c += s
                assert c == n_chunks
                return table

            in_start = spans([grp // 2, grp // 2]
                             + [grp] * (n_chunks // grp - 1))
            out_start = spans([grp] * (n_chunks // grp))

            def stage(k, i):
                v = st[i]
                g, sl = i // grp, i % grp
                c0i, siz = in_start[i]
                if k == 0:
                    if i == c0i:
                        v["xt"] = iopool.tile([P, siz * CW], bf16, tag="xt",
                                              bufs=4, name="xt")
                        nc.sync.dma_start(
                            out=v["xt"],
                            in_=x[:, c0i * CW:(c0i + siz) * CW])
                elif k == 2:
                    xt = st[c0i]["xt"]
                    v["ps1"] = psum.tile([P, CW], f32, tag="ps", name="ps")
                    for m in range(CW // MW):
                        c0 = (i - c0i) * CW + m * MW
                        mm_pair(v["ps1"][:, m * MW:(m + 1) * MW], 0,
                                xt[:, c0:c0 + MW], noload=m > 0)
                    if i == c0i + siz - 1:
                        st[c0i].pop("xt")
                elif k == 3:
                    v["q"] = midpool.tile([P, CW], bf16, tag="q", name="q")
                    # round: bf16(v + 192) == round(v) + 192 for |v| < 64
                    if i % 2 == 0:
                        nc.scalar.activation(v["q"], v.pop("ps1"),
                                             mybir.ActivationFunctionType.Copy,
                                             bias=MAGIC)
                    else:
                        nc.vector.tensor_scalar_add(v["q"], v.pop("ps1"),
                                                    MAGIC)
                elif k == 4:
                    v["ps2"] = psum.tile([P, CW], f32, tag="ps", name="ps")
                    for m in range(CW // MW):
                        mm_pair(v["ps2"][:, m * MW:(m + 1) * MW], P,
                                v["q"][:, m * MW:(m + 1) * MW], noload=m > 0)
                    v.pop("q")
                elif k == 5:
                    c0o, so = out_start[i]
                    if i == c0o:
                        v["ot"] = iopool.tile([P, so * CW], bf16, tag="ot",
                                              name="ot")
                    ot = st[c0o]["ot"]
                    dst = ot[:, (i - c0o) * CW:(i - c0o + 1) * CW]
                    if i % 2 == 0:
                        nc.vector.tensor_scalar_add(dst, v.pop("ps2"), corr)
                    else:
                        nc.scalar.activation(dst, v.pop("ps2"), ident,
                                             bias=corr)
                elif k == 6:
                    c0o, so = out_start[i]
                    if i == c0o + so - 1:
                        nc.gpsimd.dma_start(
                            out=y[:, c0o * CW:(c0o + so) * CW],
                            in_=st[c0o].pop("ot"))

            n_stages = 7
            for t in range(n_chunks + n_stages - 1):
                for k in range(n_stages - 1, -1, -1):  # deepest stage first
                    i = t - k
                    if 0 <= i < n_chunks:
                        stage(k, i)

    nc.compile()
    _BUILD_CACHE[key] = nc
    return nc


def kernel(x: np.ndarray, block_size, qp, _trace: bool = False,
           _results_out: list | None = None) -> np.ndarray:
    n = int(block_size)
    qp = int(qp)
    b, ch, h, w = x.shape
    assert b == N_CORES, f"expected batch {N_CORES}, got {b}"
    assert h % n == 0 and w % n == 0, "padding path not implemented"
    nn = n * n
    assert P % nn == 0, f"block size {n}: {nn} must divide {P}"
    G = P // nn                      # blocks stacked per partition column
    nb = ch * (h // n) * (w // n)    # blocks per core
    assert nb % G == 0
    ftot = nb // G
    assert ftot % DW == 0

    qstep = float(np.float32(2.0 ** ((qp - 4.0) / 6.0)))
    d = _dct_matrix(n)
    a = np.kron(d, d)                # [nn, nn]: coeff = a @ vec(block)
    eye_g = np.eye(G)
    wts_np = np.zeros((P, 2 * P + 1), dtype=np.float64)
    wts_np[:, 0:P] = np.kron(eye_g, a.T / qstep)   # forward-DCT stationary
    wts_np[:, P:2 * P] = np.kron(eye_g, qstep * a)  # inverse-DCT stationary
    wts_np[:, 2 * P] = -MAGIC
    consts = {"wts": np.ascontiguousarray(
        wts_np.astype(ml_dtypes.bfloat16))}

    nc = _build(ftot)

    x_np = np.asarray(x, dtype=np.float32)
    in_maps = []
    for i in range(N_CORES):
        arr = x_np[i].reshape(ch, h // n, n, w // n, n)
        arr = arr.transpose(2, 4, 0, 1, 3).reshape(nn, nb)
        arr = arr.reshape(nn, G, ftot).transpose(1, 0, 2).reshape(P, ftot)
        in_maps.append(
            {"x": np.ascontiguousarray(arr.astype(ml_dtypes.bfloat16)),
             **consts})

    res = bass_utils.run_bass_kernel_spmd(
        nc, in_maps, core_ids=list(range(N_CORES)), trace=_trace)
    if _results_out is not None:
        _results_out.append(res)

    out = np.empty((b, ch, h, w), dtype=np.float32)
    for i in range(N_CORES):
        yd = res.results[i]["y"].astype(np.float32)
        arr = yd.reshape(G, nn, ftot).transpose(1, 0, 2).reshape(nn, nb)
        arr = arr.reshape(n, n, ch, h // n, w // n)
        out[i] = arr.transpose(2, 3, 0, 4, 1).reshape(ch, h, w)
    return out
